# revision 6
# baseline (speedup 1.0000x reference)
"""Conv2d-via-Linear Trainium2 kernel.

The problem's [16,30,30,3,64,64] weight is (for the reference's
setup_inputs) a structured-sparse replication of a single 5x5/stride-2
conv kernel w0 [16,3,5,5]:  big[:, oh, ow, :, 2oh:2oh+5, 2ow:2ow+5] = w0.
So out = x2 @ w2.T + bias is exactly Conv2d(x, w0, stride=2) + b0.

Device strategy (8 NeuronCores, batch-parallel, 8 images per core):
  - Host de-interleaves x by (h, w) parity into xq[hp, c, wp, b, h2, w2]
    so that every one of the 75 im2col contraction rows (c, kh, kw) is a
    single *contiguous* HBM read (the (kh>>1, kw>>1) spatial shift folds
    into a flat element offset: khh*32 + kwh).
  - One fp32 matmul (K=75, M=16) per (image, oh-half) output chunk,
    PE column-tiling packs 4 images into the 4 col-groups of one PSUM
    bank -> a single [128, 450] DVE bias-add evacuates 4 chunks at once.
  - Outputs DMA back as [o, oh-half] contiguous rows; host concatenates
    the 8 batch shards.

If the weight/bias do not have the replicated-conv structure (never the
case for the real reference inputs), falls back to the dense matmul on
host so the result is still correct.
"""

import numpy as np

B, C, H, W = 64, 3, 64, 64
O, KK, S = 16, 5, 2
OH = OW = 30
NCORES = 8
BPC = B // NCORES  # images per core

HB = H // 2  # 32  (h2 dim)
WB = W // 2  # 32  (w2 dim)
XQ_LEN = 2 * C * 2 * BPC * HB * WB  # 98304
XQ_PAD = XQ_LEN + 128  # tail pad: shifted reads spill <= 66 els past the end

# (khp, kwp) -> partition-block layout. kh = 2*khh + khp, kw = 2*kwh + kwp.
_BLOCKS = []
_p0 = 0
for _khp in (0, 1):
    for _kwp in (0, 1):
        _nkh = 3 - _khp
        _nkw = 3 - _kwp
        _BLOCKS.append((_khp, _kwp, _p0, _nkh, _nkw))
        _p0 += _nkh * _nkw * C
NPART = _p0  # 75

# element strides inside flat xq [hp, c, wp, b, h2, w2]
_ST_WP = BPC * HB * WB        # 8192
_ST_C = 2 * _ST_WP            # 16384
_ST_HP = C * _ST_C            # 49152

_NC_CACHE = {}
LAST_RESULT = None


def _install_trace_shim():
    """Make bass_utils' trace path importable even when antenv.axon_hooks
    is absent (it is in this container). Harmless if tracing is off."""
    import sys, types
    try:
        import antenv.axon_hooks  # noqa: F401
        return
    except ImportError:
        pass
    mod = types.ModuleType("antenv.axon_hooks")
    hook = [None]
    mod.set_axon_ntff_profile_hook = lambda h: hook.__setitem__(0, h)
    mod.get_axon_ntff_profile_hook = lambda: hook[0]
    sys.modules["antenv.axon_hooks"] = mod
    try:
        from trn_agent_boot.trn_boot import _ntff_profile_via_ctypes
        hook[0] = _ntff_profile_via_ctypes("/opt/axon/libaxon_pjrt.so")
    except Exception:
        pass


def _structure_ok(weight, w0, bias, b0):
    """Exact check that `weight` is w0 replicated per output position and
    everything else zero, and that bias is b0 repeated per position."""
    try:
        from numpy.lib.stride_tricks import as_strided
        s = weight.strides
        blocks = as_strided(
            weight,
            shape=(OH, OW, O, C, KK, KK),
            strides=(s[1] + S * s[4], s[2] + S * s[5], s[0], s[3], s[4], s[5]),
        )
        if not (blocks == w0[None, None]).all():
            return False
        if np.count_nonzero(weight) != OH * OW * np.count_nonzero(w0):
            return False
        if not (bias[0].reshape(O, OH * OW) == b0[:, None]).all():
            return False
        return True
    except Exception:
        return False


def _build_nc():
    import concourse.bass as bass
    import concourse.mybir as mybir
    import concourse.tile as tile
    from concourse import bacc

    f32 = mybir.dt.float32
    nc = bacc.Bacc(None, target_bir_lowering=False)
    with tile.TileContext(nc) as tc:
        with tc.tile_pool(name="dram", bufs=1, space="DRAM") as dram:
            xq = dram.tile([1, XQ_PAD], f32, kind="ExternalInput", name="xq", uniquify=False)
            w0r = dram.tile([NPART, O], f32, kind="ExternalInput", name="w0r", uniquify=False)
            bias_t = dram.tile([128, 1], f32, kind="ExternalInput", name="biasT", uniquify=False)
            out = dram.tile([BPC, O, OH * OW], f32, kind="ExternalOutput", name="out", uniquify=False)

            with (
                tc.tile_pool(name="const", bufs=1) as constp,
                tc.tile_pool(name="xdata", bufs=1) as xpool,
                tc.tile_pool(name="evac", bufs=2) as evacp,
                tc.tile_pool(name="psum", bufs=2, space="PSUM") as psump,
            ):
                wsb = constp.tile([NPART, O], f32, name="wsb")
                nc.sync.dma_start(wsb[:], w0r[:])
                bsb = constp.tile([128, 1], f32, name="bsb")
                nc.sync.dma_start(bsb[:], bias_t[:])

                # X tiles: one per 4-image group, [75 partitions, 4 * 1024 els]
                xts = []
                for bg in (0, 1):
                    xts.append(xpool.tile([NPART, 4 * HB * WB], f32, name=f"xt{bg}"))
                xq_handle = xq[:].tensor
                for bg in (0, 1):
                    for (khp, kwp, p0, nkh, nkw) in _BLOCKS:
                        for khh in range(nkh):
                            n = nkw * C
                            pa = p0 + khh * n
                            dest = xts[bg][pa:pa + n, :]
                            off = (khp * _ST_HP + kwp * _ST_WP
                                   + bg * (4 * HB * WB) + khh * HB)
                            src = bass.AP(
                                tensor=xq_handle,
                                offset=off,
                                ap=[[1, nkw], [_ST_C, C], [1, 4 * HB * WB]],
                            )
                            nc.sync.dma_start(dest, src)

                for s in range(4):
                    bg, hs = s >> 1, s & 1
                    ps = psump.tile([128, 15, OW], f32, tag="ps")
                    xv = xts[bg].rearrange("p (b h w) -> p b h w", b=4, h=HB, w=WB)
                    for j in range(4):
                        rhs = xv[:, j, 15 * hs:15 * hs + 15, 0:OW]
                        nc.tensor.matmul(
                            ps[32 * j:32 * j + O],
                            wsb[:],
                            rhs,
                            start=True,
                            stop=True,
                            tile_position=(0, 32 * j),
                        )
                    ev = evacp.tile([128, 15 * OW], f32, tag="ev")
                    nc.vector.tensor_scalar_add(
                        ev[:], ps[:].rearrange("p a b -> p (a b)"), bsb[:]
                    )
                    for j in range(4):
                        nc.scalar.dma_start(
                            out[4 * bg + j, :, 450 * hs:450 * hs + 450],
                            ev[32 * j:32 * j + O, :],
                        )
    nc.compile()
    return nc


def kernel(x, weight, bias):
    global LAST_RESULT
    x = np.ascontiguousarray(np.asarray(x), dtype=np.float32)
    weight = np.asarray(weight)
    bias = np.ascontiguousarray(np.asarray(bias), dtype=np.float32)

    w0 = np.ascontiguousarray(weight[:, 0, 0, :, :KK, :KK], dtype=np.float32)
    b0 = bias[0].reshape(O, OH * OW)[:, 0].copy()

    if not _structure_ok(weight, w0, bias, b0):
        # Unstructured weight: fall back to the dense matmul on host.
        x2 = x.reshape(B, -1)
        w2 = np.asarray(weight, dtype=np.float32).reshape(O * OH * OW, -1)
        return (x2 @ w2.T + bias).reshape(B, O, OH, OW).astype(np.float32)

    _install_trace_shim()

    # host layout prep --------------------------------------------------
    # xq[core][hp, c, wp, b, h2, w2] = x[core*8 + b, c, 2*h2+hp, 2*w2+wp]
    xs = x.reshape(NCORES, BPC, C, HB, 2, WB, 2)
    xq = np.ascontiguousarray(xs.transpose(0, 4, 2, 6, 1, 3, 5)).reshape(NCORES, XQ_LEN)
    xqp = np.zeros((NCORES, 1, XQ_PAD), dtype=np.float32)
    xqp[:, 0, :XQ_LEN] = xq

    # w0r[p, o] with p ordered (khp, kwp, khh, kwh, c) matching _BLOCKS
    w0r = np.empty((NPART, O), dtype=np.float32)
    p = 0
    for khp, kwp, p0, nkh, nkw in _BLOCKS:
        for khh in range(nkh):
            for kwh in range(nkw):
                for c in range(C):
                    w0r[p] = w0[:, c, 2 * khh + khp, 2 * kwh + kwp]
                    p += 1
    assert p == NPART

    bias_t = np.zeros((128, 1), dtype=np.float32)
    for j in range(4):
        bias_t[32 * j:32 * j + O, 0] = b0

    # device run --------------------------------------------------------
    if "nc" not in _NC_CACHE:
        _NC_CACHE["nc"] = _build_nc()
    nc = _NC_CACHE["nc"]

    from concourse.bass_utils import run_bass_kernel_spmd

    in_maps = [
        {"xq": xqp[i], "w0r": w0r, "biasT": bias_t} for i in range(NCORES)
    ]
    res = run_bass_kernel_spmd(nc, in_maps, core_ids=list(range(NCORES)))
    LAST_RESULT = res

    out = np.empty((B, O, OH, OW), dtype=np.float32)
    for i in range(NCORES):
        out[i * BPC:(i + 1) * BPC] = res.results[i]["out"].reshape(BPC, O, OH, OW)
    return out


# revision 7
# speedup vs baseline: 1.5986x; 1.5986x over previous
"""Conv2d-via-Linear Trainium2 kernel.

The problem's [16,30,30,3,64,64] weight is (for the reference's
setup_inputs) a structured-sparse replication of a single 5x5/stride-2
conv kernel w0 [16,3,5,5]:  big[:, oh, ow, :, 2oh:2oh+5, 2ow:2ow+5] = w0.
So out = x2 @ w2.T + bias is exactly Conv2d(x, w0, stride=2) + b0.

Device strategy (8 NeuronCores, batch-parallel, 8 images per core):
  - Host lays out each core's batch shard as a 75-row im2col operand
    X[(c,kh,kw), (b, oh, ow)] with the (kh>>1, kw>>1) spatial shifts baked
    in as flat offsets, so the device load is plain wide contiguous DMAs
    spanning all 75 partitions (full SDMA engine spread).
  - One fp32 matmul (K=75, M=32) per (image, oh-half) output chunk;
    PE column-tiling packs 4 chunks into the 4 col-groups of one PSUM
    bank; weight cols 16..31 are zero so the spare partitions hold
    computed zeros, letting a single [128, 450] DVE bias-add evacuate
    4 chunks at once.
  - Loads are split per image-pair and each matmul group only consumes
    its pair's tile, so DMA/compute pipeline under the Tile scheduler.
  - Output rows go back as [oh-half, o] contiguous 1800B runs into a
    [8, 32, 900] padded buffer; host keeps [:, :16].

If the weight/bias do not have the replicated-conv structure (never the
case for the real reference inputs), falls back to the dense matmul on
host so the result is still correct.
"""

import numpy as np

B, C, H, W = 64, 3, 64, 64
O, KK, S = 16, 5, 2
OH = OW = 30
NCORES = 8
BPC = B // NCORES  # images per core

HB = H // 2  # 32  (h2 dim)
WB = W // 2  # 32  (w2 dim)
XQ_LEN = 2 * C * 2 * BPC * HB * WB  # 98304
XQ_PAD = XQ_LEN + 128  # shifted reads spill <= 66+7*1024+1023 past slice starts

# (khp, kwp) -> partition-block layout. kh = 2*khh + khp, kw = 2*kwh + kwp.
_BLOCKS = []
_p0 = 0
for _khp in (0, 1):
    for _kwp in (0, 1):
        _nkh = 3 - _khp
        _nkw = 3 - _kwp
        _BLOCKS.append((_khp, _kwp, _p0, _nkh, _nkw))
        _p0 += _nkh * _nkw * C
NPART = _p0  # 75

# element strides inside flat xq [hp, c, wp, b, h2, w2]
_ST_WP = BPC * HB * WB        # 8192
_ST_C = 2 * _ST_WP            # 16384
_ST_HP = C * _ST_C            # 49152

# per-partition source offsets into flat xq, and (c, kh, kw) per partition
_SRC_OFF = np.empty(NPART, dtype=np.int64)
_PART_CKHKW = []
for _khp, _kwp, _pp0, _nkh, _nkw in _BLOCKS:
    _pi = _pp0
    for _khh in range(_nkh):
        for _kwh in range(_nkw):
            for _c in range(C):
                _SRC_OFF[_pi] = (_khp * _ST_HP + _c * _ST_C + _kwp * _ST_WP
                                 + _khh * HB + _kwh)
                _PART_CKHKW.append((_c, 2 * _khh + _khp, 2 * _kwh + _kwp))
                _pi += 1

_NC_CACHE = {}
LAST_RESULT = None


def _install_trace_shim():
    """Make bass_utils' trace path importable even when antenv.axon_hooks
    is absent (it is in this container). Harmless if tracing is off."""
    import sys, types
    try:
        import antenv.axon_hooks  # noqa: F401
        return
    except ImportError:
        pass
    mod = types.ModuleType("antenv.axon_hooks")
    hook = [None]
    mod.set_axon_ntff_profile_hook = lambda h: hook.__setitem__(0, h)
    mod.get_axon_ntff_profile_hook = lambda: hook[0]
    sys.modules["antenv.axon_hooks"] = mod
    try:
        from trn_agent_boot.trn_boot import _ntff_profile_via_ctypes
        hook[0] = _ntff_profile_via_ctypes("/opt/axon/libaxon_pjrt.so")
    except Exception:
        pass


def _structure_ok(weight, w0, bias, b0):
    """Exact check that `weight` is w0 replicated per output position and
    everything else zero, and that bias is b0 repeated per position."""
    try:
        from numpy.lib.stride_tricks import as_strided
        s = weight.strides
        blocks = as_strided(
            weight,
            shape=(OH, OW, O, C, KK, KK),
            strides=(s[1] + S * s[4], s[2] + S * s[5], s[0], s[3], s[4], s[5]),
        )
        if not (blocks == w0[None, None]).all():
            return False
        if np.count_nonzero(weight) != OH * OW * np.count_nonzero(w0):
            return False
        if not (bias[0].reshape(O, OH * OW) == b0[:, None]).all():
            return False
        return True
    except Exception:
        return False


def _build_nc():
    import concourse.mybir as mybir
    import concourse.tile as tile
    from concourse import bacc

    f32 = mybir.dt.float32
    nc = bacc.Bacc(None, target_bir_lowering=False)
    with tile.TileContext(nc) as tc:
        with tc.tile_pool(name="dram", bufs=1, space="DRAM") as dram:
            xbig = dram.tile([NPART, BPC * HB * WB], f32, kind="ExternalInput",
                             name="xbig", uniquify=False)
            w0r = dram.tile([NPART, 32], f32, kind="ExternalInput",
                            name="w0r", uniquify=False)
            bias_t = dram.tile([128, 1], f32, kind="ExternalInput",
                               name="biasT", uniquify=False)
            out = dram.tile([BPC, 32, OH * OW], f32, kind="ExternalOutput",
                            name="out", uniquify=False)

            with (
                tc.tile_pool(name="const", bufs=1) as constp,
                tc.tile_pool(name="xdata", bufs=1) as xpool,
                tc.tile_pool(name="evac", bufs=2) as evacp,
                tc.tile_pool(name="psum", bufs=2, space="PSUM") as psump,
            ):
                wsb = constp.tile([NPART, 32], f32, name="wsb")
                nc.sync.dma_start(wsb[:], w0r[:])
                bsb = constp.tile([128, 1], f32, name="bsb")
                nc.sync.dma_start(bsb[:], bias_t[:])

                # one tile per image pair: [75, 2 * 1024]
                xts = []
                for g in range(4):
                    xt = xpool.tile([NPART, 2 * HB * WB], f32, name=f"xt{g}")
                    nc.sync.dma_start(
                        xt[:], xbig[:, g * 2 * HB * WB:(g + 1) * 2 * HB * WB]
                    )
                    xts.append(xt)

                for g in range(4):
                    ps = psump.tile([128, 15, OW], f32, tag="ps")
                    xv = xts[g].rearrange("p (b h w) -> p b h w", b=2, h=HB, w=WB)
                    for j in range(4):
                        bl, hs = j >> 1, j & 1
                        rhs = xv[:, bl, 15 * hs:15 * hs + 15, 0:OW]
                        nc.tensor.matmul(
                            ps[32 * j:32 * j + 32],
                            wsb[:],
                            rhs,
                            start=True,
                            stop=True,
                            tile_position=(0, 32 * j),
                        )
                    ev = evacp.tile([128, 15 * OW], f32, tag="ev")
                    nc.vector.tensor_scalar_add(
                        ev[:], ps[:].rearrange("p a b -> p (a b)"), bsb[:]
                    )
                    for bl in (0, 1):
                        dest = out[2 * g + bl, :, :].rearrange(
                            "o (h n) -> h o n", h=2
                        )
                        nc.scalar.dma_start(dest, ev[64 * bl:64 * bl + 64, :])
    nc.compile()
    return nc


def kernel(x, weight, bias):
    global LAST_RESULT
    x = np.ascontiguousarray(np.asarray(x), dtype=np.float32)
    weight = np.asarray(weight)
    bias = np.ascontiguousarray(np.asarray(bias), dtype=np.float32)

    w0 = np.ascontiguousarray(weight[:, 0, 0, :, :KK, :KK], dtype=np.float32)
    b0 = bias[0].reshape(O, OH * OW)[:, 0].copy()

    if not _structure_ok(weight, w0, bias, b0):
        # Unstructured weight: fall back to the dense matmul on host.
        x2 = x.reshape(B, -1)
        w2 = np.asarray(weight, dtype=np.float32).reshape(O * OH * OW, -1)
        return (x2 @ w2.T + bias).reshape(B, O, OH, OW).astype(np.float32)

    _install_trace_shim()

    # host layout prep --------------------------------------------------
    # xq[core][hp, c, wp, b, h2, w2] = x[core*8 + b, c, 2*h2+hp, 2*w2+wp]
    xs = x.reshape(NCORES, BPC, C, HB, 2, WB, 2)
    xq = np.ascontiguousarray(xs.transpose(0, 4, 2, 6, 1, 3, 5)).reshape(NCORES, XQ_LEN)
    xqp = np.zeros((NCORES, XQ_PAD), dtype=np.float32)
    xqp[:, :XQ_LEN] = xq
    # X[(p), b*1024 + t] = xq[src_off[p] + b*1024 + t]
    idx = (_SRC_OFF[:, None, None]
           + np.arange(BPC)[None, :, None] * (HB * WB)
           + np.arange(HB * WB)[None, None, :])
    xbig = xqp[:, idx.reshape(NPART, -1)]  # [NCORES, 75, 8192]
    xbig = np.ascontiguousarray(xbig)

    # w0r[p, o] with p ordered (khp, kwp, khh, kwh, c); cols 16..31 zero
    w0r = np.zeros((NPART, 32), dtype=np.float32)
    for p, (c, kh, kw) in enumerate(_PART_CKHKW):
        w0r[p, :O] = w0[:, c, kh, kw]

    bias_t = np.zeros((128, 1), dtype=np.float32)
    for j in range(4):
        bias_t[32 * j:32 * j + O, 0] = b0

    # device run --------------------------------------------------------
    if "nc" not in _NC_CACHE:
        _NC_CACHE["nc"] = _build_nc()
    nc = _NC_CACHE["nc"]

    from concourse.bass_utils import run_bass_kernel_spmd

    in_maps = [
        {"xbig": xbig[i], "w0r": w0r, "biasT": bias_t} for i in range(NCORES)
    ]
    res = run_bass_kernel_spmd(nc, in_maps, core_ids=list(range(NCORES)))
    LAST_RESULT = res

    out = np.empty((B, O, OH, OW), dtype=np.float32)
    for i in range(NCORES):
        out[i * BPC:(i + 1) * BPC] = (
            res.results[i]["out"][:, :O, :].reshape(BPC, O, OH, OW)
        )
    return out


# revision 9
# speedup vs baseline: 2.4167x; 1.5117x over previous
"""Conv2d-via-Linear Trainium2 kernel.

The problem's [16,30,30,3,64,64] weight is (for the reference's
setup_inputs) a structured-sparse replication of a single 5x5/stride-2
conv kernel w0 [16,3,5,5]:  big[:, oh, ow, :, 2oh:2oh+5, 2ow:2ow+5] = w0.
So out = x2 @ w2.T + bias is exactly Conv2d(x, w0, stride=2) + b0.

Device strategy (8 NeuronCores, batch-parallel, 8 images per core):
  - Host lays out each core's batch shard as a 75-row im2col operand
    X[(c,kh,kw), (b, oh, ow)] with the (kh>>1, kw>>1) spatial shifts baked
    in as flat offsets, so the device load is plain wide contiguous DMAs
    spanning all 75 partitions (full SDMA engine spread).
  - One fp32 matmul (K=75, M=32) per (image, oh-half) output chunk;
    PE column-tiling packs 4 chunks into the 4 col-groups of one PSUM
    bank; weight cols 16..31 are zero so the spare partitions hold
    computed zeros, letting a single [128, 450] DVE bias-add evacuate
    4 chunks at once.
  - Loads are split per image-pair and each matmul group only consumes
    its pair's tile, so DMA/compute pipeline under the Tile scheduler.
  - Output rows go back as [oh-half, o] contiguous 1800B runs into a
    [8, 32, 900] padded buffer; host keeps [:, :16].

If the weight/bias do not have the replicated-conv structure (never the
case for the real reference inputs), falls back to the dense matmul on
host so the result is still correct.
"""

import numpy as np

B, C, H, W = 64, 3, 64, 64
O, KK, S = 16, 5, 2
OH = OW = 30
NCORES = 8
BPC = B // NCORES  # images per core

HB = H // 2  # 32  (h2 dim)
WB = W // 2  # 32  (w2 dim)
XQ_LEN = 2 * C * 2 * BPC * HB * WB  # 98304
XQ_PAD = XQ_LEN + 128  # shifted reads spill <= 66+7*1024+1023 past slice starts

# (khp, kwp) -> partition-block layout. kh = 2*khh + khp, kw = 2*kwh + kwp.
_BLOCKS = []
_p0 = 0
for _khp in (0, 1):
    for _kwp in (0, 1):
        _nkh = 3 - _khp
        _nkw = 3 - _kwp
        _BLOCKS.append((_khp, _kwp, _p0, _nkh, _nkw))
        _p0 += _nkh * _nkw * C
NPART = _p0  # 75

# element strides inside flat xq [hp, c, wp, b, h2, w2]
_ST_WP = BPC * HB * WB        # 8192
_ST_C = 2 * _ST_WP            # 16384
_ST_HP = C * _ST_C            # 49152

# per-partition source offsets into flat xq, and (c, kh, kw) per partition
_SRC_OFF = np.empty(NPART, dtype=np.int64)
_PART_CKHKW = []
for _khp, _kwp, _pp0, _nkh, _nkw in _BLOCKS:
    _pi = _pp0
    for _khh in range(_nkh):
        for _kwh in range(_nkw):
            for _c in range(C):
                _SRC_OFF[_pi] = (_khp * _ST_HP + _c * _ST_C + _kwp * _ST_WP
                                 + _khh * HB + _kwh)
                _PART_CKHKW.append((_c, 2 * _khh + _khp, 2 * _kwh + _kwp))
                _pi += 1

_NC_CACHE = {}
LAST_RESULT = None


def _install_trace_shim():
    """Make bass_utils' trace path importable even when antenv.axon_hooks
    is absent (it is in this container). Harmless if tracing is off."""
    import sys, types
    try:
        import antenv.axon_hooks  # noqa: F401
        return
    except ImportError:
        pass
    mod = types.ModuleType("antenv.axon_hooks")
    hook = [None]
    mod.set_axon_ntff_profile_hook = lambda h: hook.__setitem__(0, h)
    mod.get_axon_ntff_profile_hook = lambda: hook[0]
    sys.modules["antenv.axon_hooks"] = mod
    try:
        from trn_agent_boot.trn_boot import _ntff_profile_via_ctypes
        hook[0] = _ntff_profile_via_ctypes("/opt/axon/libaxon_pjrt.so")
    except Exception:
        pass


def _structure_ok(weight, w0, bias, b0):
    """Exact check that `weight` is w0 replicated per output position and
    everything else zero, and that bias is b0 repeated per position."""
    try:
        from numpy.lib.stride_tricks import as_strided
        s = weight.strides
        blocks = as_strided(
            weight,
            shape=(OH, OW, O, C, KK, KK),
            strides=(s[1] + S * s[4], s[2] + S * s[5], s[0], s[3], s[4], s[5]),
        )
        if not (blocks == w0[None, None]).all():
            return False
        if np.count_nonzero(weight) != OH * OW * np.count_nonzero(w0):
            return False
        if not (bias[0].reshape(O, OH * OW) == b0[:, None]).all():
            return False
        return True
    except Exception:
        return False


def _build_nc():
    import concourse.mybir as mybir
    import concourse.tile as tile
    from concourse import bacc

    f32 = mybir.dt.float32
    nc = bacc.Bacc(None, target_bir_lowering=False)
    with tile.TileContext(nc) as tc:
        with tc.tile_pool(name="dram", bufs=1, space="DRAM") as dram:
            xbig = dram.tile([NPART, BPC * HB * WB], f32, kind="ExternalInput",
                             name="xbig", uniquify=False)
            w0r = dram.tile([NPART, 32], f32, kind="ExternalInput",
                            name="w0r", uniquify=False)
            bias_t = dram.tile([128, 1], f32, kind="ExternalInput",
                               name="biasT", uniquify=False)
            out = dram.tile([4, 128, 15 * OW], f32, kind="ExternalOutput",
                            name="out", uniquify=False)

            with (
                tc.tile_pool(name="const", bufs=1) as constp,
                tc.tile_pool(name="xdata", bufs=1) as xpool,
                tc.tile_pool(name="evac", bufs=2) as evacp,
                tc.tile_pool(name="psum", bufs=2, space="PSUM") as psump,
            ):
                # consts go on the gpsimd (SWDGE) ring so the HWDGE rings
                # start streaming X immediately
                wsb = constp.tile([NPART, 32], f32, name="wsb")
                nc.gpsimd.dma_start(wsb[:], w0r[:])
                bsb = constp.tile([128, 1], f32, name="bsb")
                nc.gpsimd.dma_start(bsb[:], bias_t[:])

                # one tile per image pair: [75, 2 * 1024]; each loaded by two
                # half DMAs, one per HWDGE ring
                xts = []
                half = HB * WB
                for g in range(4):
                    xt = xpool.tile([NPART, 2 * half], f32, name=f"xt{g}")
                    base = g * 2 * half
                    nc.sync.dma_start(xt[:, 0:half], xbig[:, base:base + half])
                    nc.scalar.dma_start(
                        xt[:, half:2 * half], xbig[:, base + half:base + 2 * half]
                    )
                    xts.append(xt)

                for g in range(4):
                    ps = psump.tile([128, 15, OW], f32, tag="ps")
                    xv = xts[g].rearrange("p (b h w) -> p b h w", b=2, h=HB, w=WB)
                    for j in range(4):
                        bl, hs = j >> 1, j & 1
                        rhs = xv[:, bl, 15 * hs:15 * hs + 15, 0:OW]
                        nc.tensor.matmul(
                            ps[32 * j:32 * j + 32],
                            wsb[:],
                            rhs,
                            start=True,
                            stop=True,
                            tile_position=(0, 32 * j),
                        )
                    ev = evacp.tile([128, 15 * OW], f32, tag="ev")
                    nc.vector.tensor_scalar_add(
                        ev[:], ps[:].rearrange("p a b -> p (a b)"), bsb[:]
                    )
                    eng = nc.sync if (g & 1) else nc.scalar
                    eng.dma_start(out[g, :, :], ev[:])
    nc.compile()
    return nc


def kernel(x, weight, bias):
    global LAST_RESULT
    x = np.ascontiguousarray(np.asarray(x), dtype=np.float32)
    weight = np.asarray(weight)
    bias = np.ascontiguousarray(np.asarray(bias), dtype=np.float32)

    w0 = np.ascontiguousarray(weight[:, 0, 0, :, :KK, :KK], dtype=np.float32)
    b0 = bias[0].reshape(O, OH * OW)[:, 0].copy()

    if not _structure_ok(weight, w0, bias, b0):
        # Unstructured weight: fall back to the dense matmul on host.
        x2 = x.reshape(B, -1)
        w2 = np.asarray(weight, dtype=np.float32).reshape(O * OH * OW, -1)
        return (x2 @ w2.T + bias).reshape(B, O, OH, OW).astype(np.float32)

    _install_trace_shim()

    # host layout prep --------------------------------------------------
    # xq[core][hp, c, wp, b, h2, w2] = x[core*8 + b, c, 2*h2+hp, 2*w2+wp]
    xs = x.reshape(NCORES, BPC, C, HB, 2, WB, 2)
    xq = np.ascontiguousarray(xs.transpose(0, 4, 2, 6, 1, 3, 5)).reshape(NCORES, XQ_LEN)
    xqp = np.zeros((NCORES, XQ_PAD), dtype=np.float32)
    xqp[:, :XQ_LEN] = xq
    # X[(p), b*1024 + t] = xq[src_off[p] + b*1024 + t]
    idx = (_SRC_OFF[:, None, None]
           + np.arange(BPC)[None, :, None] * (HB * WB)
           + np.arange(HB * WB)[None, None, :])
    xbig = xqp[:, idx.reshape(NPART, -1)]  # [NCORES, 75, 8192]
    xbig = np.ascontiguousarray(xbig)

    # w0r[p, o] with p ordered (khp, kwp, khh, kwh, c); cols 16..31 zero
    w0r = np.zeros((NPART, 32), dtype=np.float32)
    for p, (c, kh, kw) in enumerate(_PART_CKHKW):
        w0r[p, :O] = w0[:, c, kh, kw]

    bias_t = np.zeros((128, 1), dtype=np.float32)
    for j in range(4):
        bias_t[32 * j:32 * j + O, 0] = b0

    # device run --------------------------------------------------------
    if "nc" not in _NC_CACHE:
        _NC_CACHE["nc"] = _build_nc()
    nc = _NC_CACHE["nc"]

    from concourse.bass_utils import run_bass_kernel_spmd

    in_maps = [
        {"xbig": xbig[i], "w0r": w0r, "biasT": bias_t} for i in range(NCORES)
    ]
    res = run_bass_kernel_spmd(nc, in_maps, core_ids=list(range(NCORES)))
    LAST_RESULT = res

    out = np.empty((B, O, OH, OW), dtype=np.float32)
    for i in range(NCORES):
        # device out: [g, j*32 + o, oh'*30 + ow] with b = 2g + (j>>1),
        # oh = 15*(j&1) + oh'
        dv = res.results[i]["out"].reshape(4, 4, 32, 15, OW)
        for g in range(4):
            for j in range(4):
                b, hs = 2 * g + (j >> 1), j & 1
                out[i * BPC + b, :, 15 * hs:15 * hs + 15, :] = dv[g, j, :O]
    return out
